# revision 14
# baseline (speedup 1.0000x reference)
"""Trainium2 Bass kernel for nn_ActivationDelta.

Op (per element): at 42 fixed "activation" columns of a [50000, 2133] f32
matrix, where the value is nonzero, replace it with clip(v + delta, 0, 1);
everywhere else pass through.

Strategy (pure data parallel over rows, 8 cores; per core 6250 rows):
  - All 42 activation columns live in cols [0, 283) -- only a 288-col band
    of each row can change.
  - Full rows stream through SBUF in [128 x (G*2133)] tiles (G row-blocks
    of 128 rows per DMA => ~4.4 MB transfers). Only the 288-col band of
    each tile is touched by compute (in place):
        pred     = (x != 0) && (mask != 0)      (uint8, for CopyPredicated)
        bump     = min(max(x + delta, 0), 1)
        x[band]  = where(pred, bump, x[band])   (copy_predicated)
    then the full tile is stored to the output rows.
  - Raw bass (no Tile scheduler): loads issue on the SP HWDGE ring, stores
    on the ACT HWDGE ring, compute on DVE, with explicit semaphores. Each
    instruction carries at most ONE semaphore wait (walrus codegen limit
    on this path); extra waits are standalone wait_ge instructions.

HBM traffic per core ~= 2 x 53.3 MB -> ~300 us at ~358 GB/s/core.
"""

from contextlib import ExitStack

import numpy as np

import concourse.bass as bass
import concourse.mybir as mybir
from concourse.bass_utils import run_bass_kernel_spmd

F32 = mybir.dt.float32
U8 = mybir.dt.uint8

N_FRAMES = 50000
FEAT_DIM = 2133
N_CORES = 8
ROWS = N_FRAMES // N_CORES  # 6250 rows per core
P = 128                     # SBUF partitions
BAND = 288                  # cols [0, BAND) cover all activation columns
G = 2                       # 128-row blocks per tile (~2.2 MB per DMA)
NBUF = 6                    # x/pred/bump buffer rotation depth
FULL_BLOCKS = ROWS // P     # 48
TAIL_ROWS = ROWS - FULL_BLOCKS * P  # 106
N_GROUPS = FULL_BLOCKS // G         # 12 full groups (+1 tail group)
GB = G * BAND
GW = G * FEAT_DIM

# Activation column indices (hardcoded from the module's feature layout).
ACT_IDXS = [0, 81] + list(range(165, 283, 3))  # 42 indices, max = 282

ADD = mybir.AluOpType.add
MAX = mybir.AluOpType.max
MIN = mybir.AluOpType.min
LAND = mybir.AluOpType.logical_and


def _groups(repeat: int):
    """Yield (t, r0, rows, g) for every group of every repetition."""
    t = 0
    for _ in range(repeat):
        for i in range(N_GROUPS):
            yield t, i * G * P, G * P, G
            t += 1
        yield t, N_GROUPS * G * P, TAIL_ROWS, 1
        t += 1


def build_nc(repeat: int = 1) -> bass.Bass:
    """Single-core Bass program (SPMD: same program runs on all 8 cores).

    repeat > 1 re-runs the whole pipeline (used only for dev timing)."""
    nc = bass.Bass()
    feat = nc.declare_dram_parameter("features", [ROWS, FEAT_DIM], F32, isOutput=False)
    # consts[:, 0] = delta (replicated), consts[:, 1:] = band mask tiled G times
    consts = nc.declare_dram_parameter("consts", [P, 1 + GB], F32, isOutput=False)
    out = nc.declare_dram_parameter("out", [ROWS, FEAT_DIM], F32, isOutput=True)

    n_groups_total = repeat * (N_GROUPS + 1)

    with ExitStack() as ctx:
        block = ctx.enter_context(nc.Block())
        s_const = ctx.enter_context(nc.semaphore("s_const"))
        s_load = [
            ctx.enter_context(nc.semaphore(f"s_load{i}")) for i in range(NBUF)
        ]
        s_dve = ctx.enter_context(nc.semaphore("s_dve"))
        s_store = [
            ctx.enter_context(nc.semaphore(f"s_store{i}")) for i in range(NBUF)
        ]
        ctile = ctx.enter_context(nc.sbuf_tensor("ctile", [P, 1 + GB], F32))
        xs = [
            ctx.enter_context(nc.sbuf_tensor(f"x{i}", [P, GW], F32))
            for i in range(NBUF)
        ]
        preds = [
            ctx.enter_context(nc.sbuf_tensor(f"pred{i}", [P, GB], U8))
            for i in range(NBUF)
        ]
        bumps = [
            ctx.enter_context(nc.sbuf_tensor(f"bump{i}", [P, GB], F32))
            for i in range(NBUF)
        ]

        def feat_view(dram, r0, rows, g):
            if g == 1:
                return dram[r0 : r0 + rows, :]
            return dram[r0 : r0 + g * P, :].rearrange("(g p) c -> p g c", p=P)

        def x_full(xt, rows, g):
            if g == 1:
                return xt[:rows, 0:FEAT_DIM]
            return xt[:].rearrange("p (g c) -> p g c", g=g)

        def x_band(xt, rows, g):
            if g == 1:
                return xt[:rows, 0:BAND]
            return xt[:].rearrange("p (g c) -> p g c", g=g)[:, :, 0:BAND]

        def band3(tt, rows, g):  # pred/bump tiles: [P, G*BAND]
            if g == 1:
                return tt[:rows, 0:BAND]
            return tt[:].rearrange("p (g c) -> p g c", g=g)

        def mask_view(rows, g):
            if g == 1:
                return ctile[:rows, 1 : 1 + BAND]
            return ctile[:, 1 : 1 + GB].rearrange("p (g c) -> p g c", g=g)

        @block.sync
        def _(sp: bass.BassEngine):
            sp.dma_start(ctile[:], consts[:]).then_inc(s_const, 16)
            for t, r0, rows, g in _groups(repeat):
                if t >= NBUF:
                    # x slot's last reader is the store DMA of group t-NBUF
                    sp.wait_ge(s_store[t % NBUF], 16 * (t // NBUF))
                sp.dma_start(
                    x_full(xs[t % NBUF], rows, g), feat_view(feat, r0, rows, g)
                ).then_inc(s_load[t % NBUF], 16)

        @block.vector
        def _(v: bass.BassEngine):
            v.wait_ge(s_const, 16)
            for t, r0, rows, g in _groups(repeat):
                x = xs[t % NBUF]
                pred = preds[t % NBUF]
                bump = bumps[t % NBUF]
                v.wait_ge(s_load[t % NBUF], 16 * (t // NBUF + 1))
                np_ = P if g > 1 else rows  # partition count of the band views
                xb = x_band(x, rows, g)
                pr = band3(pred, rows, g)
                bu = band3(bump, rows, g)
                v.tensor_tensor(pr, xb, mask_view(rows, g), LAND)
                v.tensor_scalar(bu, xb, ctile[:np_, 0:1], 0.0, ADD, MAX)
                v.drain()
                v.tensor_scalar(bu, bu, 1.0, None, MIN)
                v.drain()
                # per-block 2D slices (CopyPredicated sim needs uniform views)
                last = None
                for gg in range(g):
                    last = v.copy_predicated(
                        x[:np_, gg * FEAT_DIM : gg * FEAT_DIM + BAND],
                        pred[:np_, gg * BAND : (gg + 1) * BAND],
                        bump[:np_, gg * BAND : (gg + 1) * BAND],
                    )
                last.then_inc(s_dve, 1)

        @block.scalar
        def _(act: bass.BassEngine):
            slot_counts = [0] * NBUF
            for t, r0, rows, g in _groups(repeat):
                act.wait_ge(s_dve, t + 1)
                act.dma_start(
                    feat_view(out, r0, rows, g), x_full(xs[t % NBUF], rows, g)
                ).then_inc(s_store[t % NBUF], 16)
                slot_counts[t % NBUF] += 1
            for i in range(NBUF):
                act.wait_ge(s_store[i], 16 * slot_counts[i])

    return nc


def make_in_maps(features: np.ndarray, delta: np.ndarray):
    features = np.ascontiguousarray(features, dtype=np.float32)
    d = np.float32(np.asarray(delta).reshape(-1)[0])
    mask_row = np.zeros((BAND,), dtype=np.float32)
    mask_row[ACT_IDXS] = 1.0
    consts = np.empty((P, 1 + GB), dtype=np.float32)
    consts[:, 0] = d
    consts[:, 1:] = np.tile(mask_row, G)[None, :]
    return [
        {
            "features": features[i * ROWS : (i + 1) * ROWS],
            "consts": consts,
        }
        for i in range(N_CORES)
    ]


def kernel(features: np.ndarray, delta: np.ndarray) -> np.ndarray:
    in_maps = make_in_maps(features, delta)
    nc = build_nc()
    res = run_bass_kernel_spmd(nc, in_maps, list(range(N_CORES)))
    return np.concatenate([res.results[i]["out"] for i in range(N_CORES)], axis=0)


if __name__ == "__main__":
    rng = np.random.default_rng(0)
    f = rng.random((N_FRAMES, FEAT_DIM), dtype=np.float32)
    d = np.asarray([0.03], dtype=np.float32)
    o = kernel(f, d)
    print(o.shape, o.dtype)


# revision 17
# speedup vs baseline: 1.0712x; 1.0712x over previous
"""Trainium2 Bass kernel for nn_ActivationDelta.

Op (per element): at 42 fixed "activation" columns of a [50000, 2133] f32
matrix, where the value is nonzero, replace it with clip(v + delta, 0, 1);
everywhere else pass through.

Strategy (pure data parallel over rows, 8 cores; per core 6250 rows):
  - All 42 activation columns live in cols [0, 283) -- only a 288-col band
    of each row can change.
  - Full rows stream through SBUF in [128 x (G*2133)] tiles (G row-blocks
    of 128 rows per DMA => ~2.2 MB transfers at G=2). Only the 288-col
    band of each tile is touched by compute (in place):
        pred     = (x != 0) && (mask != 0)      (uint8, for CopyPredicated)
        bump     = min(max(x + delta, 0), 1)
        x[band]  = where(pred, bump, x[band])   (copy_predicated)
    then the full tile is stored to the output rows.
  - Raw bass (no Tile scheduler): loads issue on the SP HWDGE ring, stores
    on the ACT HWDGE ring, compute on DVE, with explicit semaphores. Each
    instruction carries at most ONE semaphore wait (walrus codegen limit
    on this path); extra waits are standalone wait_ge instructions.

HBM traffic per core ~= 2 x 53.3 MB -> ~300 us at ~358 GB/s/core.
"""

import time
from contextlib import ExitStack

import numpy as np

import concourse.bass as bass
import concourse.mybir as mybir
from concourse.bass_utils import run_bass_kernel_spmd

F32 = mybir.dt.float32
U8 = mybir.dt.uint8

N_FRAMES = 50000
FEAT_DIM = 2133
N_CORES = 8
ROWS = N_FRAMES // N_CORES  # 6250 rows per core
P = 128                     # SBUF partitions
BAND = 288                  # cols [0, BAND) cover all activation columns
G = 2                       # 128-row blocks per tile (~2.2 MB per DMA)
NBUF = 6                    # x/pred/bump buffer rotation depth
FULL_BLOCKS = ROWS // P     # 48
TAIL_ROWS = ROWS - FULL_BLOCKS * P  # 106
N_GROUPS = FULL_BLOCKS // G         # 12 full groups (+1 tail group)
GB = G * BAND
GW = G * FEAT_DIM

# Activation column indices (hardcoded from the module's feature layout).
ACT_IDXS = [0, 81] + list(range(165, 283, 3))  # 42 indices, max = 282

ADD = mybir.AluOpType.add
MAX = mybir.AluOpType.max
MIN = mybir.AluOpType.min
LAND = mybir.AluOpType.logical_and


def _groups(repeat: int):
    """Yield (t, r0, rows, g) for every group of every repetition."""
    t = 0
    for _ in range(repeat):
        for i in range(N_GROUPS):
            yield t, i * G * P, G * P, G
            t += 1
        yield t, N_GROUPS * G * P, TAIL_ROWS, 1
        t += 1


def build_nc(repeat: int = 1) -> bass.Bass:
    """Single-core Bass program (SPMD: same program runs on all 8 cores).

    repeat > 1 re-runs the whole pipeline (used only for dev timing)."""
    nc = bass.Bass()
    feat = nc.declare_dram_parameter("features", [ROWS, FEAT_DIM], F32, isOutput=False)
    # consts[:, 0] = delta (replicated), consts[:, 1:] = band mask tiled G times
    consts = nc.declare_dram_parameter("consts", [P, 1 + GB], F32, isOutput=False)
    out = nc.declare_dram_parameter("out", [ROWS, FEAT_DIM], F32, isOutput=True)

    n_groups_total = repeat * (N_GROUPS + 1)

    with ExitStack() as ctx:
        block = ctx.enter_context(nc.Block())
        s_const = ctx.enter_context(nc.semaphore("s_const"))
        s_load = [
            ctx.enter_context(nc.semaphore(f"s_load{i}")) for i in range(NBUF)
        ]
        s_dve = ctx.enter_context(nc.semaphore("s_dve"))
        s_store = [
            ctx.enter_context(nc.semaphore(f"s_store{i}")) for i in range(NBUF)
        ]
        ctile = ctx.enter_context(nc.sbuf_tensor("ctile", [P, 1 + GB], F32))
        xs = [
            ctx.enter_context(nc.sbuf_tensor(f"x{i}", [P, GW], F32))
            for i in range(NBUF)
        ]
        preds = [
            ctx.enter_context(nc.sbuf_tensor(f"pred{i}", [P, GB], U8))
            for i in range(NBUF)
        ]
        bumps = [
            ctx.enter_context(nc.sbuf_tensor(f"bump{i}", [P, GB], F32))
            for i in range(NBUF)
        ]

        def feat_view(dram, r0, rows, g):
            if g == 1:
                return dram[r0 : r0 + rows, :]
            return dram[r0 : r0 + g * P, :].rearrange("(g p) c -> p g c", p=P)

        def x_full(xt, rows, g):
            if g == 1:
                return xt[:rows, 0:FEAT_DIM]
            return xt[:].rearrange("p (g c) -> p g c", g=g)

        def x_band(xt, rows, g):
            if g == 1:
                return xt[:rows, 0:BAND]
            return xt[:].rearrange("p (g c) -> p g c", g=g)[:, :, 0:BAND]

        def band3(tt, rows, g):  # pred/bump tiles: [P, G*BAND]
            if g == 1:
                return tt[:rows, 0:BAND]
            return tt[:].rearrange("p (g c) -> p g c", g=g)

        def mask_view(rows, g):
            if g == 1:
                return ctile[:rows, 1 : 1 + BAND]
            return ctile[:, 1 : 1 + GB].rearrange("p (g c) -> p g c", g=g)

        @block.sync
        def _(sp: bass.BassEngine):
            sp.dma_start(ctile[:], consts[:]).then_inc(s_const, 16)
            for t, r0, rows, g in _groups(repeat):
                if t >= NBUF:
                    # x slot's last reader is the store DMA of group t-NBUF
                    sp.wait_ge(s_store[t % NBUF], 16 * (t // NBUF))
                sp.dma_start(
                    x_full(xs[t % NBUF], rows, g), feat_view(feat, r0, rows, g)
                ).then_inc(s_load[t % NBUF], 16)

        @block.vector
        def _(v: bass.BassEngine):
            v.wait_ge(s_const, 16)
            for t, r0, rows, g in _groups(repeat):
                x = xs[t % NBUF]
                pred = preds[t % NBUF]
                bump = bumps[t % NBUF]
                v.wait_ge(s_load[t % NBUF], 16 * (t // NBUF + 1))
                np_ = P if g > 1 else rows  # partition count of the band views
                xb = x_band(x, rows, g)
                pr = band3(pred, rows, g)
                bu = band3(bump, rows, g)
                v.tensor_tensor(pr, xb, mask_view(rows, g), LAND)
                v.tensor_scalar(bu, xb, ctile[:np_, 0:1], 0.0, ADD, MAX)
                v.drain()
                v.tensor_scalar(bu, bu, 1.0, None, MIN)
                v.drain()
                # per-block 2D slices (CopyPredicated sim needs uniform views)
                last = None
                for gg in range(g):
                    last = v.copy_predicated(
                        x[:np_, gg * FEAT_DIM : gg * FEAT_DIM + BAND],
                        pred[:np_, gg * BAND : (gg + 1) * BAND],
                        bump[:np_, gg * BAND : (gg + 1) * BAND],
                    )
                last.then_inc(s_dve, 1)

        @block.scalar
        def _(act: bass.BassEngine):
            slot_counts = [0] * NBUF
            for t, r0, rows, g in _groups(repeat):
                act.wait_ge(s_dve, t + 1)
                act.dma_start(
                    feat_view(out, r0, rows, g), x_full(xs[t % NBUF], rows, g)
                ).then_inc(s_store[t % NBUF], 16)
                slot_counts[t % NBUF] += 1
            for i in range(NBUF):
                act.wait_ge(s_store[i], 16 * slot_counts[i])

    return nc


def make_in_maps(features: np.ndarray, delta: np.ndarray):
    features = np.ascontiguousarray(features, dtype=np.float32)
    d = np.float32(np.asarray(delta).reshape(-1)[0])
    mask_row = np.zeros((BAND,), dtype=np.float32)
    mask_row[ACT_IDXS] = 1.0
    consts = np.empty((P, 1 + GB), dtype=np.float32)
    consts[:, 0] = d
    consts[:, 1:] = np.tile(mask_row, G)[None, :]
    return [
        {
            "features": features[i * ROWS : (i + 1) * ROWS],
            "consts": consts,
        }
        for i in range(N_CORES)
    ]


def kernel(features: np.ndarray, delta: np.ndarray) -> np.ndarray:
    in_maps = make_in_maps(features, delta)
    nc = build_nc()
    last_err = None
    for _ in range(3):  # retry transient tunnel/runtime failures
        try:
            res = run_bass_kernel_spmd(nc, in_maps, list(range(N_CORES)))
            return np.concatenate(
                [res.results[i]["out"] for i in range(N_CORES)], axis=0
            )
        except Exception as e:  # noqa: BLE001
            last_err = e
            time.sleep(5)
    raise last_err


if __name__ == "__main__":
    rng = np.random.default_rng(0)
    f = rng.random((N_FRAMES, FEAT_DIM), dtype=np.float32)
    d = np.asarray([0.03], dtype=np.float32)
    o = kernel(f, d)
    print(o.shape, o.dtype)
